# revision 83
# baseline (speedup 1.0000x reference)
"""Trainium2 Bass kernel for nn_AttentionBlock (B=8, N=1024, DIM=768, H=12, HD=64).

Softmax is over the HEADS axis (legacy nn.Softmax(dim=None) on 4D -> dim=1),
NOT the key axis:  attn[b,h,n,m] = exp(s[b,h,n,m]) / sum_h' exp(s[b,h',n,m]).

Sharding: batch across the 8 cores (one batch element per core, zero
collectives).  All matmul operands are bf16 (1 cycle/row on the PE, any
moving size); PSUM accumulation stays fp32.  The softmax middle section
(exp / head-sum / normalize) runs in bf16, which doubles DVE tensor-op
throughput (2x_1p mode).

Per core:
  head:   q(nt0), k, v projections, chunk-major.  k/q are produced in
          [n, e] layout and DMA-transposed (xbar) into [e, n]; v stays
          [n, e].  All through a single 2-bank ping-pong PSUM pool.
  phase2: per (n-tile 256, m-chunk 128): 12 K=64 score matmuls in 3 waves
          of 4 heads into the same 2x2-bank pool; ACT exp (scale fused,
          1024-elem instrs) -> E bf16; head-sum as a bf16 tensor-tensor
          tree (DVE) + tail adds (Pool); reciprocal (DVE); normalize muls
          split DVE/Pool; attnv FLIPPED: stationary = E chunk [m,128n],
          moving = v [m,64] -> acc [128n, 64d] PSUM, M=128 (half the PE
          cycles of the [d,n] orientation).  attnv emission is software-
          pipelined one chunk behind the scores.
  phase3: acc -> outNE [n, e] bf16 -> DMA-transpose -> outT [e, n];
          y = outT.T @ W_out per 128-n-block through a 1-bank aux PSUM
          pool; bias added during PSUM evac; q projections for the next
          n-tile also ride the aux pool.
"""

import json
import os as _os

_os.environ.setdefault("BASS_NEVER_TRACE", "1")  # no NTFF hook in this env

import numpy as np
import ml_dtypes

import concourse.bass as bass
import concourse.mybir as mybir
import concourse.tile as tile
from concourse.bass_utils import run_bass_kernel_spmd
from concourse.masks import make_identity

# ----------------------------------------------------------------------------
# BIR legalizer: this container's walrus accepts at most ONE sync wait per
# instruction; Tile emits several.  Hoist excess waits onto preceding
# same-engine EventSemaphore (pure wait) instructions.
# ----------------------------------------------------------------------------


def _legalize_bir_json_bytes(data: bytes) -> bytes:
    d = json.loads(data)
    uid = [0]

    def mk_wait(engine, wait, debug):
        uid[0] += 1
        return {
            "debug": debug,
            "engine": engine,
            "ins": [],
            "name": f"I-legalize-{uid[0]}",
            "opcode": "EventSemaphore",
            "outs": [],
            "sync_info": {"on_update": [], "on_wait": [wait]},
        }

    for fn in d.get("functions", []):
        for bb in fn.get("blocks", []):
            out = []
            for inst in bb.get("instructions", []):
                si = inst.get("sync_info")
                ow = (si or {}).get("on_wait") or []
                if len(ow) > 1:
                    for w in ow[:-1]:
                        out.append(mk_wait(inst["engine"], w, inst.get("debug")))
                    si["on_wait"] = [ow[-1]]
                out.append(inst)
            bb["instructions"] = out
    return json.dumps(d).encode()


def _install_legalizer():
    if getattr(bass.Bass, "_legalize_installed", False):
        return
    orig = bass.Bass.to_json_bytes

    def patched(self, *a, **k):
        return _legalize_bir_json_bytes(orig(self, *a, **k))

    bass.Bass.to_json_bytes = patched
    bass.Bass._legalize_installed = True


_install_legalizer()

# ----------------------------------------------------------------------------
# Problem constants (hardcoded per contract)
# ----------------------------------------------------------------------------
B, N, DIM = 8, 1024, 768
HEADS, HEAD_DIM = 12, 64
INNER = HEADS * HEAD_DIM  # 768
SCALE = HEAD_DIM**-0.5
N_CORES = 8

F32 = mybir.dt.float32
BF16 = mybir.dt.bfloat16

NT = 4  # n tiles of 256
NTS = 256
MC = 8  # m chunks of 128
DC = 6  # contraction chunks of 128 (DIM/128)

MULS_DVE = int(_os.environ.get("K_MULS_DVE", "4"))  # head slots normalized on DVE
KV_HEAD = int(_os.environ.get("K_KV_HEAD", "2"))  # kv chunks emitted before phase 2

# Softmax rebase: scores are computed as D_h = s_h - s_0 (h = 1..11) via
# K=128 matmuls over stacked [q_h ; -q_0] x [k_h ; k_0] operands -- free on
# the PE (output-bound) and it eliminates head 0's exp entirely:
#   attn_0 = R',  attn_h = exp(D_h * scale) * R',  R' = 1/(1 + sum_h E'_h).
# E slot j (0..10) holds head j+1.  Score waves: slots [0:4], [4:8], [8:11].
NHE = HEADS - 1  # 11 exp'd heads
WAVE_SLOTS = [(0, 3), (3, 7), (7, 11)]


def build_nc() -> bass.Bass:
    nc = bass.Bass()
    xT_ext = nc.dram_tensor("xT", [DIM, N], BF16, kind="ExternalInput")
    wq_ext = nc.dram_tensor("w_qkv", [DIM, 3 * INNER], BF16, kind="ExternalInput")
    wo_ext = nc.dram_tensor("w_out", [INNER, DIM], BF16, kind="ExternalInput")
    bias_ext = nc.dram_tensor("bias", [128, DIM], F32, kind="ExternalInput")
    y_ext = nc.dram_tensor("y", [N, DIM], F32, kind="ExternalOutput")

    with tile.TileContext(nc) as tc:
        with (
            tc.tile_pool(name="persist", bufs=1) as persist,
            tc.tile_pool(name="stage", bufs=2) as stage,
            tc.tile_pool(name="epool", bufs=4) as epool,
            tc.tile_pool(name="small", bufs=3) as small,
            tc.tile_pool(name="outne", bufs=3) as outne,
            tc.tile_pool(name="ystage", bufs=3) as ysg,
            tc.tile_pool(name="big", bufs=2, space="PSUM") as bigp,
            tc.tile_pool(name="accp", bufs=1, space="PSUM") as accp,
            tc.tile_pool(name="auxp", bufs=1, space="PSUM") as auxp,
        ):
            xT = persist.tile([128, DC, N], BF16, tag="xT")
            wqq = persist.tile([128, DC, INNER], BF16, tag="wqq")
            wk = persist.tile([128, DC, INNER], BF16, tag="wk")
            wv = persist.tile([128, DC, INNER], BF16, tag="wv")
            wo = persist.tile([128, DC, DIM], BF16, tag="wo")
            bias = persist.tile([128, DIM], F32, tag="bias")
            qA = persist.tile([128, NHE, N], BF16, tag="qA")
            kA = persist.tile([128, NHE, N], BF16, tag="kA")
            v = persist.tile([128, MC, INNER], BF16, tag="v")
            outT = persist.tile([128, DC, N], BF16, tag="outT")
            warm = persist.tile([128, 512], BF16, tag="warm")
            warm2 = persist.tile([128, 32], BF16, tag="warm2")
            ident = persist.tile([128, 128], BF16, tag="ident")

            # ---- input DMAs (head) ----
            nc.gpsimd.memset(warm[:], 0.0)
            make_identity(nc, ident[:])
            # warmup: pay the one-time exp ACT-table load while DMAs land
            # (writes a scratch tile so the PE warmup matmuls reading `warm`
            # don't serialize behind the table load)
            nc.scalar.activation(
                warm2[:], warm[:, 0:32], mybir.ActivationFunctionType.Exp,
                scale=1.0,
            )
            # interleave the tensors the first projections need, dc-major, so
            # the q/k gens can start as soon as the first chunks land.  The
            # first scores only touch xT's first n-quarter; the rest streams
            # on the scalar queue during ACT's idle head (keeping sync clear
            # for the latency-critical stage transposes).
            for dc in range(DC):
                nc.sync.dma_start(
                    xT[:, dc, 0:256], xT_ext[dc * 128 : (dc + 1) * 128, 0:256]
                )
                nc.sync.dma_start(
                    wqq[:, dc, :], wq_ext[dc * 128 : (dc + 1) * 128, 0:INNER]
                )
                nc.gpsimd.dma_start(
                    wk[:, dc, :], wq_ext[dc * 128 : (dc + 1) * 128, INNER : 2 * INNER]
                )

            for dc in range(DC):
                nc.gpsimd.dma_start(
                    wv[:, dc, :],
                    wq_ext[dc * 128 : (dc + 1) * 128, 2 * INNER : 3 * INNER],
                )
            for dc in range(DC):
                nc.sync.dma_start(wo[:, dc, :], wo_ext[dc * 128 : (dc + 1) * 128, :])
            nc.gpsimd.dma_start(bias[:], bias_ext[:])

            # PE clock-ramp warmup: dummy matmuls while the first DMAs land
            # (the p-state model needs ~3us of continuous execution to reach
            # 2.4 GHz).  They run through the aux psum bank, which has no
            # real user until well into phase 2.
            wps = auxp.tile([128, 512], F32, tag="aux", name="warmps")
            for _ in range(5):
                nc.tensor.matmul(
                    wps[:], warm[:, 0:128], warm[:],
                    start=True, stop=True, skip_group_check=True,
                )

            # PSUM evacuations can only run on ACT/DVE (GPSIMD cannot access
            # PSUM on TRN2 hardware).  Rotate between the two; ACT has idle
            # headroom through the PE-bound head.
            evac_rr = [0]

            def head_evac(dst, src):
                e = evac_rr[0] % 2
                evac_rr[0] += 1
                if e == 0:
                    nc.scalar.copy(dst, src)
                else:
                    nc.vector.tensor_copy(dst, src)

            def proj_gen(w_tile, n0):
                """[128 n, 768 e] projection gen through the big pool.
                Returns the psum tile (caller evacs)."""
                pt = bigp.tile([128, 1024], F32, tag="big", name="pgen")
                for dc in range(DC):
                    nc.tensor.matmul(
                        pt[:, 0:512],
                        xT[:, dc, n0 : n0 + 128],
                        w_tile[:, dc, 0:512],
                        start=(dc == 0),
                        stop=(dc == DC - 1),
                    )
                    nc.tensor.matmul(
                        pt[:, 512:768],
                        xT[:, dc, n0 : n0 + 128],
                        w_tile[:, dc, 512:768],
                        start=(dc == 0),
                        stop=(dc == DC - 1),
                    )
                return pt

            def q_head_block(nb):
                """q projection for nt0 block nb -> augmented stage (slot j =
                [q_{j+1} | -q_0]) -> qA via dma transpose."""
                pt = proj_gen(wqq, nb * 128)
                qs = stage.tile([128, NHE, 128], BF16, tag="qs", name="qs")
                head_evac(
                    qs[:, :, 0:64],
                    pt[:, 64:768].rearrange("p (a b) -> p a b", a=NHE),
                )
                nc.vector.tensor_scalar_mul(
                    qs[:, :, 64:128],
                    pt[:, 0:64].unsqueeze(1).broadcast_to((128, NHE, 64)),
                    -1.0,
                )
                nc.sync.dma_start_transpose(
                    qA[:, :, nb * 128 : (nb + 1) * 128],
                    qs[:].rearrange("p a b -> p (a b)"),
                )

            def k_block(mc):
                pt = proj_gen(wk, mc * 128)
                ks = stage.tile([128, NHE, 128], BF16, tag="ks", name="ks")
                head_evac(
                    ks[:, :, 0:64],
                    pt[:, 64:768].rearrange("p (a b) -> p a b", a=NHE),
                )
                head_evac(
                    ks[:, :, 64:128],
                    pt[:, 0:64].unsqueeze(1).broadcast_to((128, NHE, 64)),
                )
                nc.sync.dma_start_transpose(
                    kA[:, :, mc * 128 : (mc + 1) * 128],
                    ks[:].rearrange("p a b -> p (a b)"),
                )

            def v_block(mc):
                pt2 = proj_gen(wv, mc * 128)
                head_evac(v[:, mc, :], pt2[:, 0:768])

            def kv_block(mc):
                k_block(mc)
                v_block(mc)

            q_head_block(0)
            q_head_block(1)
            for dc in range(DC):
                nc.scalar.dma_start(
                    xT[:, dc, 256:1024],
                    xT_ext[dc * 128 : (dc + 1) * 128, 256:1024],
                )
            for mc in range(min(KV_HEAD, MC)):
                kv_block(mc)
            kv_jobs = list(range(min(KV_HEAD, MC), MC))

            # ---------------- phase 2 + overlapped phase 1/3 ----------------
            acc = [None, None, None]  # acc psum tiles for current nt

            def emit_attnv_bank(prev, bank, nbs=(0, 1)):
                """attnv for chunk `prev` = (nt, mc, E, R), PSUM bank `bank`.
                Head 0's numerator is the constant 1, so its stationary is
                R' itself; head h>=1 uses E slot h-1."""
                pmc, E_, R_ = prev[1], prev[2], prev[3]
                for i in range(4):
                    h = 4 * bank + i
                    for nb in nbs:
                        first = pmc == 0 and i == 0 and nb == 0
                        stat = (
                            R_[:, nb * 128 : (nb + 1) * 128]
                            if h == 0
                            else E_[:, h - 1, nb * 128 : (nb + 1) * 128]
                        )
                        nc.tensor.matmul(
                            acc[bank][:, (h % 4) + 4 * nb, :],
                            stat,
                            v[:, pmc, h * 64 : (h + 1) * 64],
                            start=first,
                            stop=False,
                            skip_group_check=True,
                        )

            def emit_acc_evac_nb(nt_, nb, tail=False):
                """acc -> outNE (bf16) -> DMA-transpose -> outT, one n-block."""
                one = outne.tile([128, INNER], BF16, tag=f"outNE{nb}", name="one")
                engines = (
                    [nc.scalar.copy, nc.vector.tensor_copy, nc.scalar.copy]
                    if tail
                    else [nc.vector.tensor_copy, nc.vector.tensor_copy,
                          nc.vector.tensor_copy]
                )
                for q in range(3):
                    engines[q](
                        one[:, q * 256 : (q + 1) * 256],
                        acc[q][:, 4 * nb : 4 * nb + 4, :],
                    )
                nc.sync.dma_start_transpose(
                    outT[:, :, nt_ * NTS + nb * 128 : nt_ * NTS + (nb + 1) * 128],
                    one[:],
                )

            def emit_acc_evac(nt_):
                for nb in range(2):
                    emit_acc_evac_nb(nt_, nb)

            def emit_tail_out_nb(nt_, nb):
                """tail variant: acc -> outNE -> PE transpose -> outT.  The
                xbar DMA's ~3.3us dispatch+transfer+sem latency is on the
                critical path here, so spend 768 PE cycles instead."""
                n0 = nt_ * NTS + nb * 128
                one = outne.tile([128, INNER], BF16, tag=f"outNE{nb}", name="one")
                for q, eng in enumerate(
                    [nc.scalar.copy, nc.vector.tensor_copy, nc.scalar.copy]
                ):
                    eng(one[:, q * 256 : (q + 1) * 256],
                        acc[q][:, 4 * nb : 4 * nb + 4, :])
                pt = bigp.tile([128, 1024], F32, tag="big", name="ttp")
                ptb = pt[:].bitcast(BF16)  # [128, 2048] bf16 view
                for ec in range(DC):
                    nc.tensor.matmul(
                        ptb[:, ec * 128 : (ec + 1) * 128],
                        one[:, ec * 128 : (ec + 1) * 128],
                        ident[:],
                        is_transpose=True,
                        skip_group_check=True,
                    )
                nc.vector.tensor_copy(
                    outT[:, 0:3, n0 : n0 + 128],
                    ptb[:, 0:384].rearrange("p (a b) -> p a b", a=3),
                )
                nc.scalar.copy(
                    outT[:, 3:6, n0 : n0 + 128],
                    ptb[:, 384:768].rearrange("p (a b) -> p a b", a=3),
                )

            def big_y_job(nt_, nb):
                """tail output projection through the (now free) big pool."""
                pt = bigp.tile([128, 1024], F32, tag="big", name="ytail")
                n0 = nt_ * NTS + nb * 128
                for ec in range(DC):
                    nc.tensor.matmul(
                        pt[:, 0:512],
                        outT[:, ec, n0 : n0 + 128],
                        wo[:, ec, 0:512],
                        start=(ec == 0),
                        stop=(ec == DC - 1),
                    )
                    nc.tensor.matmul(
                        pt[:, 512:768],
                        outT[:, ec, n0 : n0 + 128],
                        wo[:, ec, 512:768],
                        start=(ec == 0),
                        stop=(ec == DC - 1),
                    )
                ys = ysg.tile([128, DIM], F32, tag="yst", name="yst")
                nc.vector.tensor_tensor(
                    ys[:, 0:512], pt[:, 0:512], bias[:, 0:512], op=mybir.AluOpType.add
                )
                nc.vector.tensor_tensor(
                    ys[:, 512:768], pt[:, 512:768], bias[:, 512:768],
                    op=mybir.AluOpType.add,
                )
                nc.sync.dma_start(y_ext[n0 : n0 + 128, 0:512], ys[:, 0:512])
                nc.sync.dma_start(y_ext[n0 : n0 + 128, 512:768], ys[:, 512:768])

            aux_qs = {}  # nb -> staged q tile awaiting its second half

            def aux_q_job(nt_, nb, half):
                """q projection for n-tile nt_, block nb, e-half `half` through
                the 1-bank aux pool.  half 0 = e 0:512, half 1 = e 512:768."""
                w0, wid = (0, 512) if half == 0 else (512, 256)
                pa = auxp.tile([128, 512], F32, tag="aux", name="aq")
                n0 = nt_ * NTS + nb * 128
                for dc in range(DC):
                    nc.tensor.matmul(
                        pa[:, 0:wid],
                        xT[:, dc, n0 : n0 + 128],
                        wqq[:, dc, w0 : w0 + wid],
                        start=(dc == 0),
                        stop=(dc == DC - 1),
                    )
                if half == 0:
                    aux_qs[nb] = stage.tile(
                        [128, NHE, 128], BF16, tag=f"aqs{nb}", name="aqs", bufs=2
                    )
                qs = aux_qs[nb]
                if half == 0:
                    # heads 1..7 (psum cols 64:512) into slot tops 0..6, and
                    # the negated q_0 broadcast into all slot bottoms
                    nc.vector.tensor_copy(
                        qs[:, 0:7, 0:64],
                        pa[:, 64:512].rearrange("p (a b) -> p a b", a=7),
                    )
                    nc.vector.tensor_scalar_mul(
                        qs[:, :, 64:128],
                        pa[:, 0:64].unsqueeze(1).broadcast_to((128, NHE, 64)),
                        -1.0,
                    )
                else:
                    # heads 8..11 (psum cols 0:256 of the 512:768 half)
                    nc.vector.tensor_copy(
                        qs[:, 7:11, 0:64],
                        pa[:, 0:256].rearrange("p (a b) -> p a b", a=4),
                    )
                    nc.sync.dma_start_transpose(
                        qA[:, :, n0 : n0 + 128],
                        qs[:].rearrange("p a b -> p (a b)"),
                    )

            def aux_y_job(nt_, nb, half):
                """output projection y rows [nt_*256+nb*128, +128), dim-half."""
                w0, wid = (0, 512) if half == 0 else (512, 256)
                pa = auxp.tile([128, 512], F32, tag="aux", name="ay")
                n0 = nt_ * NTS + nb * 128
                for ec in range(DC):
                    nc.tensor.matmul(
                        pa[:, 0:wid],
                        outT[:, ec, n0 : n0 + 128],
                        wo[:, ec, w0 : w0 + wid],
                        start=(ec == 0),
                        stop=(ec == DC - 1),
                    )
                ys = ysg.tile([128, 512], F32, tag="ys", name="ys")
                nc.vector.tensor_tensor(
                    ys[:, 0:wid], pa[:, 0:wid], bias[:, w0 : w0 + wid],
                    op=mybir.AluOpType.add,
                )
                nc.sync.dma_start(y_ext[n0 : n0 + 128, w0 : w0 + wid], ys[:, 0:wid])

            prev = None  # (nt, mc, E, started) of previous chunk
            for nt in range(NT):
                for mc in range(MC):
                    if prev is None or prev[0] != nt:
                        # new n-tile: allocate fresh acc gens (waits on the
                        # previous nt's evac readers via tag cycling)
                        for q in range(3):
                            acc[q] = accp.tile(
                                [128, 8, 64], F32, tag=f"acc{q}", name=f"acc{q}"
                            )
                    E = epool.tile([128, NHE, NTS], BF16, tag="E", name="E")
                    for w, (s0, s1) in enumerate(WAVE_SLOTS):
                        nw = s1 - s0
                        sc = bigp.tile([128, 1024], F32, tag="big", name="sc")
                        for s in range(nw):
                            j = s0 + s
                            nc.tensor.matmul(
                                sc[:, s * 256 : (s + 1) * 256],
                                kA[:, j, mc * 128 : (mc + 1) * 128],
                                qA[:, j, nt * NTS : (nt + 1) * NTS],
                                start=True,
                                stop=True,
                            )
                        nc.scalar.activation(
                            E[:, s0:s1, :],
                            sc[:, 0 : nw * 256].rearrange("p (s n) -> p s n", s=nw),
                            mybir.ActivationFunctionType.Exp,
                            scale=float(SCALE),
                        )
                        if prev is not None:
                            emit_attnv_bank(prev, w)
                    # head-sum tree: DVE bf16 2x for the wide steps
                    t6 = small.tile([128, 6, NTS], BF16, tag="t6", name="t6")
                    t3 = small.tile([128, 3, NTS], BF16, tag="t3", name="t3")
                    S = small.tile([128, NTS], BF16, tag="S", name="S")
                    R = small.tile([128, NTS], BF16, tag="R", name="R")
                    add = mybir.AluOpType.add
                    if nt == NT - 1 and mc == MC - 1:
                        # final chunk: split by n-halves so n-block 0 retires
                        # with minimum latency (half A entirely on DVE, half B
                        # tree on Pool) -- this chain heads the drain sequence
                        ca, cb = slice(0, 128), slice(128, 256)
                        nc.vector.tensor_tensor(
                            t6[:, 0:5, ca], E[:, 0:5, ca], E[:, 5:10, ca], op=add
                        )
                        nc.vector.tensor_tensor(
                            t3[:, 0:2, ca], t6[:, 0:2, ca], t6[:, 2:4, ca], op=add
                        )
                        nc.vector.tensor_tensor(
                            t3[:, 2, ca], t3[:, 0, ca], t3[:, 1, ca], op=add
                        )
                        nc.vector.tensor_tensor(
                            t3[:, 2, ca], t3[:, 2, ca], t6[:, 4, ca], op=add
                        )
                        nc.vector.tensor_tensor(
                            t3[:, 2, ca], t3[:, 2, ca], E[:, 10, ca], op=add
                        )
                        nc.vector.tensor_scalar_add(S[:, ca], t3[:, 2, ca], 1.0)
                        with nc.allow_low_precision(reason="softmax denom"):
                            nc.vector.reciprocal(R[:, ca], S[:, ca])
                        nc.vector.tensor_mul(
                            E[:, :, ca],
                            E[:, :, ca],
                            R[:, ca].unsqueeze(1).broadcast_to((128, NHE, 128)),
                        )
                        nc.gpsimd.tensor_add(
                            t6[:, 0:5, cb], E[:, 0:5, cb], E[:, 5:10, cb]
                        )
                        nc.gpsimd.tensor_add(
                            t3[:, 0:2, cb], t6[:, 0:2, cb], t6[:, 2:4, cb]
                        )
                        nc.gpsimd.tensor_add(t3[:, 2, cb], t3[:, 0, cb], t3[:, 1, cb])
                        nc.gpsimd.tensor_add(t3[:, 2, cb], t3[:, 2, cb], t6[:, 4, cb])
                        nc.gpsimd.tensor_add(t3[:, 2, cb], t3[:, 2, cb], E[:, 10, cb])
                        nc.gpsimd.tensor_scalar_add(S[:, cb], t3[:, 2, cb], 1.0)
                        with nc.allow_low_precision(reason="softmax denom"):
                            nc.vector.reciprocal(R[:, cb], S[:, cb])
                        nc.gpsimd.tensor_mul(
                            E[:, :, cb],
                            E[:, :, cb],
                            R[:, cb].unsqueeze(1).broadcast_to((128, NHE, 128)),
                        )
                    else:
                        nc.vector.tensor_tensor(
                            t6[:, 0:5, :], E[:, 0:5, :], E[:, 5:10, :], op=add
                        )
                        nc.vector.tensor_tensor(
                            t3[:, 0:2, :], t6[:, 0:2, :], t6[:, 2:4, :], op=add
                        )
                        nc.gpsimd.tensor_add(t3[:, 2, :], t3[:, 0, :], t3[:, 1, :])
                        nc.gpsimd.tensor_add(t3[:, 2, :], t3[:, 2, :], t6[:, 4, :])
                        nc.gpsimd.tensor_add(t3[:, 2, :], t3[:, 2, :], E[:, 10, :])
                        nc.gpsimd.tensor_scalar_add(S[:], t3[:, 2, :], 1.0)
                        with nc.allow_low_precision(reason="softmax denom"):
                            nc.vector.reciprocal(R[:], S[:])
                        # second-to-last chunk: all muls on DVE so a greedy
                        # Pool-queue ordering can't head-of-line block the
                        # final attnv flush
                        nd = NHE if (nt == NT - 1 and mc == MC - 2) else MULS_DVE
                        nc.vector.tensor_mul(
                            E[:, 0:nd, :],
                            E[:, 0:nd, :],
                            R[:].unsqueeze(1).broadcast_to((128, nd, NTS)),
                        )
                        if nd < NHE:
                            nc.gpsimd.tensor_mul(
                                E[:, nd:NHE, :],
                                E[:, nd:NHE, :],
                                R[:].unsqueeze(1).broadcast_to(
                                    (128, NHE - nd, NTS)
                                ),
                            )

                    # overlapped phase-1 kv production (just-in-time mode)
                    if kv_jobs and nt == 0 and kv_jobs[0] <= mc + 2:
                        kv_block(kv_jobs.pop(0))

                    # aux-pool job: q for nt+1 at mc 0..3, y for nt-1 at mc 4..7
                    if mc < 4:
                        if nt + 1 < NT:
                            aux_q_job(nt + 1, mc // 2, mc % 2)
                    else:
                        if nt > 0:
                            aux_y_job(nt - 1, (mc - 4) // 2, mc % 2)

                    prev = (nt, mc, E, R)

                # end of n-tile: flush last chunk's attnv, evac accs
                if nt < NT - 1:
                    for w in range(3):
                        emit_attnv_bank(prev, w)
                    emit_acc_evac(nt)
                else:
                    # tail: nb-ordered drain, PE transposes, big-pool y
                    for nb in range(2):
                        for w in range(3):
                            emit_attnv_bank(prev, w, nbs=(nb,))
                        emit_tail_out_nb(nt, nb)
                        big_y_job(nt, nb)
                prev = None

    return nc


_NC_CACHE = {}


def _get_nc():
    key = (MULS_DVE, KV_HEAD)
    if key not in _NC_CACHE:
        _NC_CACHE[key] = build_nc()
    return _NC_CACHE[key]


def _bf16(a: np.ndarray) -> np.ndarray:
    return np.ascontiguousarray(a.astype(ml_dtypes.bfloat16))


def kernel(x, w_qkv, w_out, b_out):
    x = np.asarray(x, dtype=np.float32)
    w_qkv = _bf16(np.asarray(w_qkv, dtype=np.float32))
    w_out = _bf16(np.asarray(w_out, dtype=np.float32))
    b_out = np.asarray(b_out, dtype=np.float32)
    bias_bc = np.ascontiguousarray(np.broadcast_to(b_out[None, :], (128, DIM)))

    nc = _get_nc()
    in_maps = []
    for b in range(B):
        in_maps.append(
            {
                "xT": _bf16(x[b].T),
                "w_qkv": w_qkv,
                "w_out": w_out,
                "bias": bias_bc,
            }
        )
    res = run_bass_kernel_spmd(nc, in_maps, list(range(N_CORES)))
    y = np.stack([res.results[i]["y"] for i in range(N_CORES)], axis=0)
    return y


# revision 85
# speedup vs baseline: 1.0082x; 1.0082x over previous
"""Trainium2 Bass kernel for nn_AttentionBlock (B=8, N=1024, DIM=768, H=12, HD=64).

Softmax is over the HEADS axis (legacy nn.Softmax(dim=None) on 4D -> dim=1),
NOT the key axis:  attn[b,h,n,m] = exp(s[b,h,n,m]) / sum_h' exp(s[b,h',n,m]).

Sharding: batch across the 8 cores (one batch element per core, zero
collectives).  All matmul operands are bf16 (1 cycle/row on the PE, any
moving size); PSUM accumulation stays fp32.  The softmax middle section
(exp / head-sum / normalize) runs in bf16, which doubles DVE tensor-op
throughput (2x_1p mode).

Per core:
  head:   q(nt0), k, v projections, chunk-major.  k/q are produced in
          [n, e] layout and DMA-transposed (xbar) into [e, n]; v stays
          [n, e].  All through a single 2-bank ping-pong PSUM pool.
  phase2: per (n-tile 256, m-chunk 128): 12 K=64 score matmuls in 3 waves
          of 4 heads into the same 2x2-bank pool; ACT exp (scale fused,
          1024-elem instrs) -> E bf16; head-sum as a bf16 tensor-tensor
          tree (DVE) + tail adds (Pool); reciprocal (DVE); normalize muls
          split DVE/Pool; attnv FLIPPED: stationary = E chunk [m,128n],
          moving = v [m,64] -> acc [128n, 64d] PSUM, M=128 (half the PE
          cycles of the [d,n] orientation).  attnv emission is software-
          pipelined one chunk behind the scores.
  phase3: acc -> outNE [n, e] bf16 -> DMA-transpose -> outT [e, n];
          y = outT.T @ W_out per 128-n-block through a 1-bank aux PSUM
          pool; bias added during PSUM evac; q projections for the next
          n-tile also ride the aux pool.
"""

import json
import os as _os

_os.environ.setdefault("BASS_NEVER_TRACE", "1")  # no NTFF hook in this env

import numpy as np
import ml_dtypes

import concourse.bass as bass
import concourse.mybir as mybir
import concourse.tile as tile
from concourse.bass_utils import run_bass_kernel_spmd
from concourse.masks import make_identity

# ----------------------------------------------------------------------------
# BIR legalizer: this container's walrus accepts at most ONE sync wait per
# instruction; Tile emits several.  Hoist excess waits onto preceding
# same-engine EventSemaphore (pure wait) instructions.
# ----------------------------------------------------------------------------


def _legalize_bir_json_bytes(data: bytes) -> bytes:
    d = json.loads(data)
    uid = [0]

    def mk_wait(engine, wait, debug):
        uid[0] += 1
        return {
            "debug": debug,
            "engine": engine,
            "ins": [],
            "name": f"I-legalize-{uid[0]}",
            "opcode": "EventSemaphore",
            "outs": [],
            "sync_info": {"on_update": [], "on_wait": [wait]},
        }

    for fn in d.get("functions", []):
        for bb in fn.get("blocks", []):
            out = []
            for inst in bb.get("instructions", []):
                si = inst.get("sync_info")
                ow = (si or {}).get("on_wait") or []
                if len(ow) > 1:
                    for w in ow[:-1]:
                        out.append(mk_wait(inst["engine"], w, inst.get("debug")))
                    si["on_wait"] = [ow[-1]]
                out.append(inst)
            bb["instructions"] = out
    return json.dumps(d).encode()


def _install_legalizer():
    if getattr(bass.Bass, "_legalize_installed", False):
        return
    orig = bass.Bass.to_json_bytes

    def patched(self, *a, **k):
        return _legalize_bir_json_bytes(orig(self, *a, **k))

    bass.Bass.to_json_bytes = patched
    bass.Bass._legalize_installed = True


_install_legalizer()

# ----------------------------------------------------------------------------
# Problem constants (hardcoded per contract)
# ----------------------------------------------------------------------------
B, N, DIM = 8, 1024, 768
HEADS, HEAD_DIM = 12, 64
INNER = HEADS * HEAD_DIM  # 768
SCALE = HEAD_DIM**-0.5
N_CORES = 8

F32 = mybir.dt.float32
BF16 = mybir.dt.bfloat16

NT = 4  # n tiles of 256
NTS = 256
MC = 8  # m chunks of 128
DC = 6  # contraction chunks of 128 (DIM/128)

MULS_DVE = int(_os.environ.get("K_MULS_DVE", "4"))  # head slots normalized on DVE
KV_HEAD = int(_os.environ.get("K_KV_HEAD", "2"))  # kv chunks emitted before phase 2

# Softmax rebase: scores are computed as D_h = s_h - s_0 (h = 1..11) via
# K=128 matmuls over stacked [q_h ; -q_0] x [k_h ; k_0] operands -- free on
# the PE (output-bound) and it eliminates head 0's exp entirely:
#   attn_0 = R',  attn_h = exp(D_h * scale) * R',  R' = 1/(1 + sum_h E'_h).
# E slot j (0..10) holds head j+1.  Score waves: slots [0:4], [4:8], [8:11].
NHE = HEADS - 1  # 11 exp'd heads
WAVE_SLOTS = [(0, 4), (4, 8), (8, 11)]


def build_nc() -> bass.Bass:
    nc = bass.Bass()
    xT_ext = nc.dram_tensor("xT", [DIM, N], BF16, kind="ExternalInput")
    wq_ext = nc.dram_tensor("w_qkv", [DIM, 3 * INNER], BF16, kind="ExternalInput")
    wo_ext = nc.dram_tensor("w_out", [INNER, DIM], BF16, kind="ExternalInput")
    bias_ext = nc.dram_tensor("bias", [128, DIM], F32, kind="ExternalInput")
    y_ext = nc.dram_tensor("y", [N, DIM], F32, kind="ExternalOutput")

    with tile.TileContext(nc) as tc:
        with (
            tc.tile_pool(name="persist", bufs=1) as persist,
            tc.tile_pool(name="stage", bufs=2) as stage,
            tc.tile_pool(name="epool", bufs=4) as epool,
            tc.tile_pool(name="small", bufs=3) as small,
            tc.tile_pool(name="outne", bufs=3) as outne,
            tc.tile_pool(name="ystage", bufs=3) as ysg,
            tc.tile_pool(name="big", bufs=2, space="PSUM") as bigp,
            tc.tile_pool(name="accp", bufs=1, space="PSUM") as accp,
            tc.tile_pool(name="auxp", bufs=1, space="PSUM") as auxp,
        ):
            xT = persist.tile([128, DC, N], BF16, tag="xT")
            wqq = persist.tile([128, DC, INNER], BF16, tag="wqq")
            wk = persist.tile([128, DC, INNER], BF16, tag="wk")
            wv = persist.tile([128, DC, INNER], BF16, tag="wv")
            wo = persist.tile([128, DC, DIM], BF16, tag="wo")
            bias = persist.tile([128, DIM], F32, tag="bias")
            qA = persist.tile([128, NHE, N], BF16, tag="qA")
            kA = persist.tile([128, NHE, N], BF16, tag="kA")
            v = persist.tile([128, MC, INNER], BF16, tag="v")
            outT = persist.tile([128, DC, N], BF16, tag="outT")
            warm = persist.tile([128, 512], BF16, tag="warm")
            warm2 = persist.tile([128, 32], BF16, tag="warm2")
            ident = persist.tile([128, 128], BF16, tag="ident")

            # ---- input DMAs (head) ----
            nc.gpsimd.memset(warm[:], 0.0)
            make_identity(nc, ident[:])
            # warmup: pay the one-time exp ACT-table load while DMAs land
            # (writes a scratch tile so the PE warmup matmuls reading `warm`
            # don't serialize behind the table load)
            nc.scalar.activation(
                warm2[:], warm[:, 0:32], mybir.ActivationFunctionType.Exp,
                scale=1.0,
            )
            # interleave the tensors the first projections need, dc-major, so
            # the q/k gens can start as soon as the first chunks land.  The
            # first scores only touch xT's first n-quarter; the rest streams
            # on the scalar queue during ACT's idle head (keeping sync clear
            # for the latency-critical stage transposes).
            for dc in range(DC):
                nc.sync.dma_start(
                    xT[:, dc, 0:512], xT_ext[dc * 128 : (dc + 1) * 128, 0:512]
                )
                nc.sync.dma_start(
                    wqq[:, dc, :], wq_ext[dc * 128 : (dc + 1) * 128, 0:INNER]
                )
                nc.gpsimd.dma_start(
                    wk[:, dc, :], wq_ext[dc * 128 : (dc + 1) * 128, INNER : 2 * INNER]
                )

            for dc in range(DC):
                nc.gpsimd.dma_start(
                    wv[:, dc, :],
                    wq_ext[dc * 128 : (dc + 1) * 128, 2 * INNER : 3 * INNER],
                )
            for dc in range(DC):
                nc.sync.dma_start(wo[:, dc, :], wo_ext[dc * 128 : (dc + 1) * 128, :])
            nc.gpsimd.dma_start(bias[:], bias_ext[:])

            # PE clock-ramp warmup: dummy matmuls while the first DMAs land
            # (the p-state model needs ~3us of continuous execution to reach
            # 2.4 GHz).  They run through the aux psum bank, which has no
            # real user until well into phase 2.
            wps = auxp.tile([128, 512], F32, tag="aux", name="warmps")
            for _ in range(5):
                nc.tensor.matmul(
                    wps[:], warm[:, 0:128], warm[:],
                    start=True, stop=True, skip_group_check=True,
                )

            # PSUM evacuations can only run on ACT/DVE (GPSIMD cannot access
            # PSUM on TRN2 hardware).  Rotate between the two; ACT has idle
            # headroom through the PE-bound head.
            evac_rr = [0]

            def head_evac(dst, src):
                e = evac_rr[0] % 2
                evac_rr[0] += 1
                if e == 0:
                    nc.scalar.copy(dst, src)
                else:
                    nc.vector.tensor_copy(dst, src)

            def proj_gen(w_tile, n0):
                """[128 n, 768 e] projection gen through the big pool.
                Returns the psum tile (caller evacs)."""
                pt = bigp.tile([128, 1024], F32, tag="big", name="pgen")
                for dc in range(DC):
                    nc.tensor.matmul(
                        pt[:, 0:512],
                        xT[:, dc, n0 : n0 + 128],
                        w_tile[:, dc, 0:512],
                        start=(dc == 0),
                        stop=(dc == DC - 1),
                    )
                    nc.tensor.matmul(
                        pt[:, 512:768],
                        xT[:, dc, n0 : n0 + 128],
                        w_tile[:, dc, 512:768],
                        start=(dc == 0),
                        stop=(dc == DC - 1),
                    )
                return pt

            def q_head_block(nb):
                """q projection for nt0 block nb -> augmented stage (slot j =
                [q_{j+1} | -q_0]) -> qA via dma transpose."""
                pt = proj_gen(wqq, nb * 128)
                qs = stage.tile([128, NHE, 128], BF16, tag="qs", name="qs")
                head_evac(
                    qs[:, :, 0:64],
                    pt[:, 64:768].rearrange("p (a b) -> p a b", a=NHE),
                )
                nc.vector.tensor_scalar_mul(
                    qs[:, :, 64:128],
                    pt[:, 0:64].unsqueeze(1).broadcast_to((128, NHE, 64)),
                    -1.0,
                )
                nc.sync.dma_start_transpose(
                    qA[:, :, nb * 128 : (nb + 1) * 128],
                    qs[:].rearrange("p a b -> p (a b)"),
                )

            def k_block(mc):
                pt = proj_gen(wk, mc * 128)
                ks = stage.tile([128, NHE, 128], BF16, tag="ks", name="ks")
                head_evac(
                    ks[:, :, 0:64],
                    pt[:, 64:768].rearrange("p (a b) -> p a b", a=NHE),
                )
                head_evac(
                    ks[:, :, 64:128],
                    pt[:, 0:64].unsqueeze(1).broadcast_to((128, NHE, 64)),
                )
                nc.sync.dma_start_transpose(
                    kA[:, :, mc * 128 : (mc + 1) * 128],
                    ks[:].rearrange("p a b -> p (a b)"),
                )

            def v_block(mc):
                pt2 = proj_gen(wv, mc * 128)
                head_evac(v[:, mc, :], pt2[:, 0:768])

            def kv_block(mc):
                k_block(mc)
                v_block(mc)

            q_head_block(0)
            q_head_block(1)
            for dc in range(DC):
                nc.scalar.dma_start(
                    xT[:, dc, 512:1024],
                    xT_ext[dc * 128 : (dc + 1) * 128, 512:1024],
                )
            for mc in range(min(KV_HEAD, MC)):
                kv_block(mc)
            kv_jobs = list(range(min(KV_HEAD, MC), MC))

            # ---------------- phase 2 + overlapped phase 1/3 ----------------
            acc = [None, None, None]  # acc psum tiles for current nt

            def emit_attnv_bank(prev, bank, nbs=(0, 1)):
                """attnv for chunk `prev` = (nt, mc, E, R), PSUM bank `bank`.
                Head 0's numerator is the constant 1, so its stationary is
                R' itself; head h>=1 uses E slot h-1."""
                pmc, E_, R_ = prev[1], prev[2], prev[3]
                for i in range(4):
                    h = 4 * bank + i
                    for nb in nbs:
                        first = pmc == 0 and i == 0 and nb == 0
                        stat = (
                            R_[:, nb * 128 : (nb + 1) * 128]
                            if h == 0
                            else E_[:, h - 1, nb * 128 : (nb + 1) * 128]
                        )
                        nc.tensor.matmul(
                            acc[bank][:, (h % 4) + 4 * nb, :],
                            stat,
                            v[:, pmc, h * 64 : (h + 1) * 64],
                            start=first,
                            stop=False,
                            skip_group_check=True,
                        )

            def emit_acc_evac_nb(nt_, nb, tail=False):
                """acc -> outNE (bf16) -> DMA-transpose -> outT, one n-block."""
                one = outne.tile([128, INNER], BF16, tag=f"outNE{nb}", name="one")
                engines = (
                    [nc.scalar.copy, nc.vector.tensor_copy, nc.scalar.copy]
                    if tail
                    else [nc.vector.tensor_copy, nc.vector.tensor_copy,
                          nc.vector.tensor_copy]
                )
                for q in range(3):
                    engines[q](
                        one[:, q * 256 : (q + 1) * 256],
                        acc[q][:, 4 * nb : 4 * nb + 4, :],
                    )
                nc.sync.dma_start_transpose(
                    outT[:, :, nt_ * NTS + nb * 128 : nt_ * NTS + (nb + 1) * 128],
                    one[:],
                )

            def emit_acc_evac(nt_):
                for nb in range(2):
                    emit_acc_evac_nb(nt_, nb)

            def emit_tail_out_nb(nt_, nb):
                """tail variant: acc -> outNE -> PE transpose -> outT.  The
                xbar DMA's ~3.3us dispatch+transfer+sem latency is on the
                critical path here, so spend 768 PE cycles instead."""
                n0 = nt_ * NTS + nb * 128
                one = outne.tile([128, INNER], BF16, tag=f"outNE{nb}", name="one")
                for q, eng in enumerate(
                    [nc.scalar.copy, nc.vector.tensor_copy, nc.scalar.copy]
                ):
                    eng(one[:, q * 256 : (q + 1) * 256],
                        acc[q][:, 4 * nb : 4 * nb + 4, :])
                pt = bigp.tile([128, 1024], F32, tag="big", name="ttp")
                ptb = pt[:].bitcast(BF16)  # [128, 2048] bf16 view
                for ec in range(DC):
                    nc.tensor.matmul(
                        ptb[:, ec * 128 : (ec + 1) * 128],
                        one[:, ec * 128 : (ec + 1) * 128],
                        ident[:],
                        is_transpose=True,
                        skip_group_check=True,
                    )
                nc.vector.tensor_copy(
                    outT[:, 0:3, n0 : n0 + 128],
                    ptb[:, 0:384].rearrange("p (a b) -> p a b", a=3),
                )
                nc.scalar.copy(
                    outT[:, 3:6, n0 : n0 + 128],
                    ptb[:, 384:768].rearrange("p (a b) -> p a b", a=3),
                )

            def big_y_job(nt_, nb):
                """tail output projection through the (now free) big pool."""
                pt = bigp.tile([128, 1024], F32, tag="big", name="ytail")
                n0 = nt_ * NTS + nb * 128
                for ec in range(DC):
                    nc.tensor.matmul(
                        pt[:, 0:512],
                        outT[:, ec, n0 : n0 + 128],
                        wo[:, ec, 0:512],
                        start=(ec == 0),
                        stop=(ec == DC - 1),
                    )
                    nc.tensor.matmul(
                        pt[:, 512:768],
                        outT[:, ec, n0 : n0 + 128],
                        wo[:, ec, 512:768],
                        start=(ec == 0),
                        stop=(ec == DC - 1),
                    )
                ys = ysg.tile([128, DIM], F32, tag="yst", name="yst")
                nc.vector.tensor_tensor(
                    ys[:, 0:512], pt[:, 0:512], bias[:, 0:512], op=mybir.AluOpType.add
                )
                nc.vector.tensor_tensor(
                    ys[:, 512:768], pt[:, 512:768], bias[:, 512:768],
                    op=mybir.AluOpType.add,
                )
                nc.sync.dma_start(y_ext[n0 : n0 + 128, 0:512], ys[:, 0:512])
                nc.sync.dma_start(y_ext[n0 : n0 + 128, 512:768], ys[:, 512:768])

            aux_qs = {}  # nb -> staged q tile awaiting its second half

            def aux_q_job(nt_, nb, half):
                """q projection for n-tile nt_, block nb, e-half `half` through
                the 1-bank aux pool.  half 0 = e 0:512, half 1 = e 512:768."""
                w0, wid = (0, 512) if half == 0 else (512, 256)
                pa = auxp.tile([128, 512], F32, tag="aux", name="aq")
                n0 = nt_ * NTS + nb * 128
                for dc in range(DC):
                    nc.tensor.matmul(
                        pa[:, 0:wid],
                        xT[:, dc, n0 : n0 + 128],
                        wqq[:, dc, w0 : w0 + wid],
                        start=(dc == 0),
                        stop=(dc == DC - 1),
                    )
                if half == 0:
                    aux_qs[nb] = stage.tile(
                        [128, NHE, 128], BF16, tag=f"aqs{nb}", name="aqs", bufs=2
                    )
                qs = aux_qs[nb]
                if half == 0:
                    # heads 1..7 (psum cols 64:512) into slot tops 0..6, and
                    # the negated q_0 broadcast into all slot bottoms
                    nc.vector.tensor_copy(
                        qs[:, 0:7, 0:64],
                        pa[:, 64:512].rearrange("p (a b) -> p a b", a=7),
                    )
                    nc.vector.tensor_scalar_mul(
                        qs[:, :, 64:128],
                        pa[:, 0:64].unsqueeze(1).broadcast_to((128, NHE, 64)),
                        -1.0,
                    )
                else:
                    # heads 8..11 (psum cols 0:256 of the 512:768 half)
                    nc.vector.tensor_copy(
                        qs[:, 7:11, 0:64],
                        pa[:, 0:256].rearrange("p (a b) -> p a b", a=4),
                    )
                    nc.sync.dma_start_transpose(
                        qA[:, :, n0 : n0 + 128],
                        qs[:].rearrange("p a b -> p (a b)"),
                    )

            def aux_y_job(nt_, nb, half):
                """output projection y rows [nt_*256+nb*128, +128), dim-half."""
                w0, wid = (0, 512) if half == 0 else (512, 256)
                pa = auxp.tile([128, 512], F32, tag="aux", name="ay")
                n0 = nt_ * NTS + nb * 128
                for ec in range(DC):
                    nc.tensor.matmul(
                        pa[:, 0:wid],
                        outT[:, ec, n0 : n0 + 128],
                        wo[:, ec, w0 : w0 + wid],
                        start=(ec == 0),
                        stop=(ec == DC - 1),
                    )
                ys = ysg.tile([128, 512], F32, tag="ys", name="ys")
                nc.vector.tensor_tensor(
                    ys[:, 0:wid], pa[:, 0:wid], bias[:, w0 : w0 + wid],
                    op=mybir.AluOpType.add,
                )
                nc.sync.dma_start(y_ext[n0 : n0 + 128, w0 : w0 + wid], ys[:, 0:wid])

            prev = None  # (nt, mc, E, started) of previous chunk
            for nt in range(NT):
                for mc in range(MC):
                    if prev is None or prev[0] != nt:
                        # new n-tile: allocate fresh acc gens (waits on the
                        # previous nt's evac readers via tag cycling)
                        for q in range(3):
                            acc[q] = accp.tile(
                                [128, 8, 64], F32, tag=f"acc{q}", name=f"acc{q}"
                            )
                    E = epool.tile([128, NHE, NTS], BF16, tag="E", name="E")
                    for w, (s0, s1) in enumerate(WAVE_SLOTS):
                        nw = s1 - s0
                        sc = bigp.tile([128, 1024], F32, tag="big", name="sc")
                        for s in range(nw):
                            j = s0 + s
                            nc.tensor.matmul(
                                sc[:, s * 256 : (s + 1) * 256],
                                kA[:, j, mc * 128 : (mc + 1) * 128],
                                qA[:, j, nt * NTS : (nt + 1) * NTS],
                                start=True,
                                stop=True,
                            )
                        nc.scalar.activation(
                            E[:, s0:s1, :],
                            sc[:, 0 : nw * 256].rearrange("p (s n) -> p s n", s=nw),
                            mybir.ActivationFunctionType.Exp,
                            scale=float(SCALE),
                        )
                        if prev is not None:
                            emit_attnv_bank(prev, w)
                    # head-sum tree: DVE bf16 2x for the wide steps
                    t6 = small.tile([128, 6, NTS], BF16, tag="t6", name="t6")
                    t3 = small.tile([128, 3, NTS], BF16, tag="t3", name="t3")
                    S = small.tile([128, NTS], BF16, tag="S", name="S")
                    R = small.tile([128, NTS], BF16, tag="R", name="R")
                    add = mybir.AluOpType.add
                    if nt == NT - 1 and mc == MC - 1:
                        # final chunk: split by n-halves so n-block 0 retires
                        # with minimum latency (half A entirely on DVE, half B
                        # tree on Pool) -- this chain heads the drain sequence
                        ca, cb = slice(0, 128), slice(128, 256)
                        nc.vector.tensor_tensor(
                            t6[:, 0:5, ca], E[:, 0:5, ca], E[:, 5:10, ca], op=add
                        )
                        nc.vector.tensor_tensor(
                            t3[:, 0:2, ca], t6[:, 0:2, ca], t6[:, 2:4, ca], op=add
                        )
                        nc.vector.tensor_tensor(
                            t3[:, 2, ca], t3[:, 0, ca], t3[:, 1, ca], op=add
                        )
                        nc.vector.tensor_tensor(
                            t3[:, 2, ca], t3[:, 2, ca], t6[:, 4, ca], op=add
                        )
                        nc.vector.tensor_tensor(
                            t3[:, 2, ca], t3[:, 2, ca], E[:, 10, ca], op=add
                        )
                        nc.vector.tensor_scalar_add(S[:, ca], t3[:, 2, ca], 1.0)
                        with nc.allow_low_precision(reason="softmax denom"):
                            nc.vector.reciprocal(R[:, ca], S[:, ca])
                        nc.vector.tensor_mul(
                            E[:, :, ca],
                            E[:, :, ca],
                            R[:, ca].unsqueeze(1).broadcast_to((128, NHE, 128)),
                        )
                        nc.gpsimd.tensor_add(
                            t6[:, 0:5, cb], E[:, 0:5, cb], E[:, 5:10, cb]
                        )
                        nc.gpsimd.tensor_add(
                            t3[:, 0:2, cb], t6[:, 0:2, cb], t6[:, 2:4, cb]
                        )
                        nc.gpsimd.tensor_add(t3[:, 2, cb], t3[:, 0, cb], t3[:, 1, cb])
                        nc.gpsimd.tensor_add(t3[:, 2, cb], t3[:, 2, cb], t6[:, 4, cb])
                        nc.gpsimd.tensor_add(t3[:, 2, cb], t3[:, 2, cb], E[:, 10, cb])
                        nc.gpsimd.tensor_scalar_add(S[:, cb], t3[:, 2, cb], 1.0)
                        with nc.allow_low_precision(reason="softmax denom"):
                            nc.vector.reciprocal(R[:, cb], S[:, cb])
                        nc.gpsimd.tensor_mul(
                            E[:, :, cb],
                            E[:, :, cb],
                            R[:, cb].unsqueeze(1).broadcast_to((128, NHE, 128)),
                        )
                    else:
                        nc.vector.tensor_tensor(
                            t6[:, 0:5, :], E[:, 0:5, :], E[:, 5:10, :], op=add
                        )
                        nc.vector.tensor_tensor(
                            t3[:, 0:2, :], t6[:, 0:2, :], t6[:, 2:4, :], op=add
                        )
                        nc.gpsimd.tensor_add(t3[:, 2, :], t3[:, 0, :], t3[:, 1, :])
                        nc.gpsimd.tensor_add(t3[:, 2, :], t3[:, 2, :], t6[:, 4, :])
                        nc.gpsimd.tensor_add(t3[:, 2, :], t3[:, 2, :], E[:, 10, :])
                        nc.gpsimd.tensor_scalar_add(S[:], t3[:, 2, :], 1.0)
                        with nc.allow_low_precision(reason="softmax denom"):
                            nc.vector.reciprocal(R[:], S[:])
                        # second-to-last chunk: all muls on DVE so a greedy
                        # Pool-queue ordering can't head-of-line block the
                        # final attnv flush
                        nd = NHE if (nt == NT - 1 and mc == MC - 2) else MULS_DVE
                        nc.vector.tensor_mul(
                            E[:, 0:nd, :],
                            E[:, 0:nd, :],
                            R[:].unsqueeze(1).broadcast_to((128, nd, NTS)),
                        )
                        if nd < NHE:
                            nc.gpsimd.tensor_mul(
                                E[:, nd:NHE, :],
                                E[:, nd:NHE, :],
                                R[:].unsqueeze(1).broadcast_to(
                                    (128, NHE - nd, NTS)
                                ),
                            )

                    # overlapped phase-1 kv production (just-in-time mode)
                    if kv_jobs and nt == 0 and kv_jobs[0] <= mc + 2:
                        kv_block(kv_jobs.pop(0))

                    # aux-pool job: q for nt+1 at mc 0..3, y for nt-1 at mc 4..7
                    if mc < 4:
                        if nt + 1 < NT:
                            aux_q_job(nt + 1, mc // 2, mc % 2)
                    else:
                        if nt > 0:
                            aux_y_job(nt - 1, (mc - 4) // 2, mc % 2)

                    prev = (nt, mc, E, R)

                # end of n-tile: flush last chunk's attnv, evac accs
                if nt < NT - 1:
                    for w in range(3):
                        emit_attnv_bank(prev, w)
                    emit_acc_evac(nt)
                else:
                    # tail: nb-ordered drain, PE transposes, big-pool y
                    for nb in range(2):
                        for w in range(3):
                            emit_attnv_bank(prev, w, nbs=(nb,))
                        emit_tail_out_nb(nt, nb)
                        big_y_job(nt, nb)
                prev = None

    return nc


_NC_CACHE = {}


def _get_nc():
    key = (MULS_DVE, KV_HEAD)
    if key not in _NC_CACHE:
        _NC_CACHE[key] = build_nc()
    return _NC_CACHE[key]


def _bf16(a: np.ndarray) -> np.ndarray:
    return np.ascontiguousarray(a.astype(ml_dtypes.bfloat16))


def kernel(x, w_qkv, w_out, b_out):
    x = np.asarray(x, dtype=np.float32)
    w_qkv = _bf16(np.asarray(w_qkv, dtype=np.float32))
    w_out = _bf16(np.asarray(w_out, dtype=np.float32))
    b_out = np.asarray(b_out, dtype=np.float32)
    bias_bc = np.ascontiguousarray(np.broadcast_to(b_out[None, :], (128, DIM)))

    nc = _get_nc()
    in_maps = []
    for b in range(B):
        in_maps.append(
            {
                "xT": _bf16(x[b].T),
                "w_qkv": w_qkv,
                "w_out": w_out,
                "bias": bias_bc,
            }
        )
    res = run_bass_kernel_spmd(nc, in_maps, list(range(N_CORES)))
    y = np.stack([res.results[i]["y"] for i in range(N_CORES)], axis=0)
    return y


# revision 87
# speedup vs baseline: 1.0089x; 1.0007x over previous
"""Trainium2 Bass kernel for nn_AttentionBlock (B=8, N=1024, DIM=768, H=12, HD=64).

Softmax is over the HEADS axis (legacy nn.Softmax(dim=None) on 4D -> dim=1),
NOT the key axis:  attn[b,h,n,m] = exp(s[b,h,n,m]) / sum_h' exp(s[b,h',n,m]).

Sharding: batch across the 8 cores (one batch element per core, zero
collectives).  All matmul operands are bf16 (1 cycle/row on the PE, any
moving size); PSUM accumulation stays fp32.  The softmax middle section
(exp / head-sum / normalize) runs in bf16, which doubles DVE tensor-op
throughput (2x_1p mode).

Per core:
  head:   q(nt0), k, v projections, chunk-major.  k/q are produced in
          [n, e] layout and DMA-transposed (xbar) into [e, n]; v stays
          [n, e].  All through a single 2-bank ping-pong PSUM pool.
  phase2: per (n-tile 256, m-chunk 128): 12 K=64 score matmuls in 3 waves
          of 4 heads into the same 2x2-bank pool; ACT exp (scale fused,
          1024-elem instrs) -> E bf16; head-sum as a bf16 tensor-tensor
          tree (DVE) + tail adds (Pool); reciprocal (DVE); normalize muls
          split DVE/Pool; attnv FLIPPED: stationary = E chunk [m,128n],
          moving = v [m,64] -> acc [128n, 64d] PSUM, M=128 (half the PE
          cycles of the [d,n] orientation).  attnv emission is software-
          pipelined one chunk behind the scores.
  phase3: acc -> outNE [n, e] bf16 -> DMA-transpose -> outT [e, n];
          y = outT.T @ W_out per 128-n-block through a 1-bank aux PSUM
          pool; bias added during PSUM evac; q projections for the next
          n-tile also ride the aux pool.
"""

import json
import os as _os

_os.environ.setdefault("BASS_NEVER_TRACE", "1")  # no NTFF hook in this env

import numpy as np
import ml_dtypes

import concourse.bass as bass
import concourse.mybir as mybir
import concourse.tile as tile
from concourse.bass_utils import run_bass_kernel_spmd
from concourse.masks import make_identity

# ----------------------------------------------------------------------------
# BIR legalizer: this container's walrus accepts at most ONE sync wait per
# instruction; Tile emits several.  Hoist excess waits onto preceding
# same-engine EventSemaphore (pure wait) instructions.
# ----------------------------------------------------------------------------


def _legalize_bir_json_bytes(data: bytes) -> bytes:
    d = json.loads(data)
    uid = [0]

    def mk_wait(engine, wait, debug):
        uid[0] += 1
        return {
            "debug": debug,
            "engine": engine,
            "ins": [],
            "name": f"I-legalize-{uid[0]}",
            "opcode": "EventSemaphore",
            "outs": [],
            "sync_info": {"on_update": [], "on_wait": [wait]},
        }

    for fn in d.get("functions", []):
        for bb in fn.get("blocks", []):
            out = []
            for inst in bb.get("instructions", []):
                si = inst.get("sync_info")
                ow = (si or {}).get("on_wait") or []
                if len(ow) > 1:
                    for w in ow[:-1]:
                        out.append(mk_wait(inst["engine"], w, inst.get("debug")))
                    si["on_wait"] = [ow[-1]]
                out.append(inst)
            bb["instructions"] = out
    return json.dumps(d).encode()


def _install_legalizer():
    if getattr(bass.Bass, "_legalize_installed", False):
        return
    orig = bass.Bass.to_json_bytes

    def patched(self, *a, **k):
        return _legalize_bir_json_bytes(orig(self, *a, **k))

    bass.Bass.to_json_bytes = patched
    bass.Bass._legalize_installed = True


_install_legalizer()

# ----------------------------------------------------------------------------
# Problem constants (hardcoded per contract)
# ----------------------------------------------------------------------------
B, N, DIM = 8, 1024, 768
HEADS, HEAD_DIM = 12, 64
INNER = HEADS * HEAD_DIM  # 768
SCALE = HEAD_DIM**-0.5
N_CORES = 8

F32 = mybir.dt.float32
BF16 = mybir.dt.bfloat16

NT = 4  # n tiles of 256
NTS = 256
MC = 8  # m chunks of 128
DC = 6  # contraction chunks of 128 (DIM/128)

MULS_DVE = int(_os.environ.get("K_MULS_DVE", "4"))  # head slots normalized on DVE
KV_HEAD = int(_os.environ.get("K_KV_HEAD", "2"))  # kv chunks emitted before phase 2

# Softmax rebase: scores are computed as D_h = s_h - s_0 (h = 1..11) via
# K=128 matmuls over stacked [q_h ; -q_0] x [k_h ; k_0] operands -- free on
# the PE (output-bound) and it eliminates head 0's exp entirely:
#   attn_0 = R',  attn_h = exp(D_h * scale) * R',  R' = 1/(1 + sum_h E'_h).
# E slot j (0..10) holds head j+1.  Score waves: slots [0:4], [4:8], [8:11].
NHE = HEADS - 1  # 11 exp'd heads
WAVE_SLOTS = [(0, 4), (4, 8), (8, 11)]


def build_nc() -> bass.Bass:
    nc = bass.Bass()
    xT_ext = nc.dram_tensor("xT", [DIM, N], BF16, kind="ExternalInput")
    wq_ext = nc.dram_tensor("w_qkv", [DIM, 3 * INNER], BF16, kind="ExternalInput")
    wo_ext = nc.dram_tensor("w_out", [INNER, DIM], BF16, kind="ExternalInput")
    bias_ext = nc.dram_tensor("bias", [128, DIM], F32, kind="ExternalInput")
    y_ext = nc.dram_tensor("y", [N, DIM], F32, kind="ExternalOutput")

    with tile.TileContext(nc) as tc:
        with (
            tc.tile_pool(name="persist", bufs=1) as persist,
            tc.tile_pool(name="stage", bufs=2) as stage,
            tc.tile_pool(name="epool", bufs=4) as epool,
            tc.tile_pool(name="small", bufs=3) as small,
            tc.tile_pool(name="outne", bufs=3) as outne,
            tc.tile_pool(name="ystage", bufs=3) as ysg,
            tc.tile_pool(name="big", bufs=2, space="PSUM") as bigp,
            tc.tile_pool(name="accp", bufs=1, space="PSUM") as accp,
            tc.tile_pool(name="auxp", bufs=1, space="PSUM") as auxp,
        ):
            xT = persist.tile([128, DC, N], BF16, tag="xT")
            wqq = persist.tile([128, DC, INNER], BF16, tag="wqq")
            wk = persist.tile([128, DC, INNER], BF16, tag="wk")
            wv = persist.tile([128, DC, INNER], BF16, tag="wv")
            wo = persist.tile([128, DC, DIM], BF16, tag="wo")
            bias = persist.tile([128, DIM], F32, tag="bias")
            qA = persist.tile([128, NHE, N], BF16, tag="qA")
            kA = persist.tile([128, NHE, N], BF16, tag="kA")
            v = persist.tile([128, MC, INNER], BF16, tag="v")
            outT = persist.tile([128, DC, N], BF16, tag="outT")
            warm = persist.tile([128, 512], BF16, tag="warm")
            warm2 = persist.tile([128, 32], BF16, tag="warm2")
            ident = persist.tile([128, 128], BF16, tag="ident")

            # ---- input DMAs (head) ----
            nc.gpsimd.memset(warm[:], 0.0)
            make_identity(nc, ident[:])
            # warmup: pay the one-time exp ACT-table load while DMAs land
            # (writes a scratch tile so the PE warmup matmuls reading `warm`
            # don't serialize behind the table load)
            nc.scalar.activation(
                warm2[:], warm[:, 0:32], mybir.ActivationFunctionType.Exp,
                scale=1.0,
            )
            # interleave the tensors the first projections need, dc-major, so
            # the q/k gens can start as soon as the first chunks land.  The
            # first scores only touch xT's first n-quarter; the rest streams
            # on the scalar queue during ACT's idle head (keeping sync clear
            # for the latency-critical stage transposes).
            for dc in range(DC):
                nc.sync.dma_start(
                    xT[:, dc, 0:256], xT_ext[dc * 128 : (dc + 1) * 128, 0:256]
                )
                nc.sync.dma_start(
                    wqq[:, dc, :], wq_ext[dc * 128 : (dc + 1) * 128, 0:INNER]
                )
                nc.gpsimd.dma_start(
                    wk[:, dc, :], wq_ext[dc * 128 : (dc + 1) * 128, INNER : 2 * INNER]
                )

            for dc in range(DC):
                nc.gpsimd.dma_start(
                    wv[:, dc, :],
                    wq_ext[dc * 128 : (dc + 1) * 128, 2 * INNER : 3 * INNER],
                )
            for dc in range(DC):
                nc.sync.dma_start(wo[:, dc, :], wo_ext[dc * 128 : (dc + 1) * 128, :])
            nc.gpsimd.dma_start(bias[:], bias_ext[:])

            # PE clock-ramp warmup: dummy matmuls while the first DMAs land
            # (the p-state model needs ~3us of continuous execution to reach
            # 2.4 GHz).  They run through the aux psum bank, which has no
            # real user until well into phase 2.
            wps = auxp.tile([128, 512], F32, tag="aux", name="warmps")
            for _ in range(4):
                nc.tensor.matmul(
                    wps[:], warm[:, 0:128], warm[:],
                    start=True, stop=True, skip_group_check=True,
                )

            # PSUM evacuations can only run on ACT/DVE (GPSIMD cannot access
            # PSUM on TRN2 hardware).  Rotate between the two; ACT has idle
            # headroom through the PE-bound head.
            evac_rr = [0]

            def head_evac(dst, src):
                e = evac_rr[0] % 2
                evac_rr[0] += 1
                if e == 0:
                    nc.scalar.copy(dst, src)
                else:
                    nc.vector.tensor_copy(dst, src)

            def proj_gen(w_tile, n0):
                """[128 n, 768 e] projection gen through the big pool.
                Returns the psum tile (caller evacs)."""
                pt = bigp.tile([128, 1024], F32, tag="big", name="pgen")
                for dc in range(DC):
                    nc.tensor.matmul(
                        pt[:, 0:512],
                        xT[:, dc, n0 : n0 + 128],
                        w_tile[:, dc, 0:512],
                        start=(dc == 0),
                        stop=(dc == DC - 1),
                    )
                    nc.tensor.matmul(
                        pt[:, 512:768],
                        xT[:, dc, n0 : n0 + 128],
                        w_tile[:, dc, 512:768],
                        start=(dc == 0),
                        stop=(dc == DC - 1),
                    )
                return pt

            def q_head_block(nb):
                """q projection for nt0 block nb -> augmented stage (slot j =
                [q_{j+1} | -q_0]) -> qA via dma transpose."""
                pt = proj_gen(wqq, nb * 128)
                qs = stage.tile([128, NHE, 128], BF16, tag="qs", name="qs")
                head_evac(
                    qs[:, :, 0:64],
                    pt[:, 64:768].rearrange("p (a b) -> p a b", a=NHE),
                )
                nc.vector.tensor_scalar_mul(
                    qs[:, :, 64:128],
                    pt[:, 0:64].unsqueeze(1).broadcast_to((128, NHE, 64)),
                    -1.0,
                )
                nc.sync.dma_start_transpose(
                    qA[:, :, nb * 128 : (nb + 1) * 128],
                    qs[:].rearrange("p a b -> p (a b)"),
                )

            def k_block(mc):
                pt = proj_gen(wk, mc * 128)
                ks = stage.tile([128, NHE, 128], BF16, tag="ks", name="ks")
                head_evac(
                    ks[:, :, 0:64],
                    pt[:, 64:768].rearrange("p (a b) -> p a b", a=NHE),
                )
                head_evac(
                    ks[:, :, 64:128],
                    pt[:, 0:64].unsqueeze(1).broadcast_to((128, NHE, 64)),
                )
                nc.sync.dma_start_transpose(
                    kA[:, :, mc * 128 : (mc + 1) * 128],
                    ks[:].rearrange("p a b -> p (a b)"),
                )

            def v_block(mc):
                pt2 = proj_gen(wv, mc * 128)
                head_evac(v[:, mc, :], pt2[:, 0:768])

            def kv_block(mc):
                k_block(mc)
                v_block(mc)

            q_head_block(0)
            q_head_block(1)
            for dc in range(DC):
                nc.scalar.dma_start(
                    xT[:, dc, 256:1024],
                    xT_ext[dc * 128 : (dc + 1) * 128, 256:1024],
                )
            for mc in range(min(KV_HEAD, MC)):
                kv_block(mc)
            kv_jobs = list(range(min(KV_HEAD, MC), MC))

            # ---------------- phase 2 + overlapped phase 1/3 ----------------
            acc = [None, None, None]  # acc psum tiles for current nt

            def emit_attnv_bank(prev, bank, nbs=(0, 1)):
                """attnv for chunk `prev` = (nt, mc, E, R), PSUM bank `bank`.
                Head 0's numerator is the constant 1, so its stationary is
                R' itself; head h>=1 uses E slot h-1."""
                pmc, E_, R_ = prev[1], prev[2], prev[3]
                for i in range(4):
                    h = 4 * bank + i
                    for nb in nbs:
                        first = pmc == 0 and i == 0 and nb == 0
                        stat = (
                            R_[:, nb * 128 : (nb + 1) * 128]
                            if h == 0
                            else E_[:, h - 1, nb * 128 : (nb + 1) * 128]
                        )
                        nc.tensor.matmul(
                            acc[bank][:, (h % 4) + 4 * nb, :],
                            stat,
                            v[:, pmc, h * 64 : (h + 1) * 64],
                            start=first,
                            stop=False,
                            skip_group_check=True,
                        )

            def emit_acc_evac_nb(nt_, nb, tail=False):
                """acc -> outNE (bf16) -> DMA-transpose -> outT, one n-block."""
                one = outne.tile([128, INNER], BF16, tag=f"outNE{nb}", name="one")
                engines = (
                    [nc.scalar.copy, nc.vector.tensor_copy, nc.scalar.copy]
                    if tail
                    else [nc.vector.tensor_copy, nc.vector.tensor_copy,
                          nc.vector.tensor_copy]
                )
                for q in range(3):
                    engines[q](
                        one[:, q * 256 : (q + 1) * 256],
                        acc[q][:, 4 * nb : 4 * nb + 4, :],
                    )
                nc.sync.dma_start_transpose(
                    outT[:, :, nt_ * NTS + nb * 128 : nt_ * NTS + (nb + 1) * 128],
                    one[:],
                )

            def emit_acc_evac(nt_):
                for nb in range(2):
                    emit_acc_evac_nb(nt_, nb)

            def emit_tail_out_nb(nt_, nb):
                """tail variant: acc -> outNE -> PE transpose -> outT.  The
                xbar DMA's ~3.3us dispatch+transfer+sem latency is on the
                critical path here, so spend 768 PE cycles instead."""
                n0 = nt_ * NTS + nb * 128
                one = outne.tile([128, INNER], BF16, tag=f"outNE{nb}", name="one")
                for q, eng in enumerate(
                    [nc.scalar.copy, nc.vector.tensor_copy, nc.scalar.copy]
                ):
                    eng(one[:, q * 256 : (q + 1) * 256],
                        acc[q][:, 4 * nb : 4 * nb + 4, :])
                pt = bigp.tile([128, 1024], F32, tag="big", name="ttp")
                ptb = pt[:].bitcast(BF16)  # [128, 2048] bf16 view
                for ec in range(DC):
                    nc.tensor.matmul(
                        ptb[:, ec * 128 : (ec + 1) * 128],
                        one[:, ec * 128 : (ec + 1) * 128],
                        ident[:],
                        is_transpose=True,
                        skip_group_check=True,
                    )
                nc.vector.tensor_copy(
                    outT[:, 0:3, n0 : n0 + 128],
                    ptb[:, 0:384].rearrange("p (a b) -> p a b", a=3),
                )
                nc.scalar.copy(
                    outT[:, 3:6, n0 : n0 + 128],
                    ptb[:, 384:768].rearrange("p (a b) -> p a b", a=3),
                )

            def big_y_job(nt_, nb):
                """tail output projection through the (now free) big pool."""
                pt = bigp.tile([128, 1024], F32, tag="big", name="ytail")
                n0 = nt_ * NTS + nb * 128
                for ec in range(DC):
                    nc.tensor.matmul(
                        pt[:, 0:512],
                        outT[:, ec, n0 : n0 + 128],
                        wo[:, ec, 0:512],
                        start=(ec == 0),
                        stop=(ec == DC - 1),
                    )
                    nc.tensor.matmul(
                        pt[:, 512:768],
                        outT[:, ec, n0 : n0 + 128],
                        wo[:, ec, 512:768],
                        start=(ec == 0),
                        stop=(ec == DC - 1),
                    )
                ys = ysg.tile([128, DIM], F32, tag="yst", name="yst")
                nc.vector.tensor_tensor(
                    ys[:, 0:512], pt[:, 0:512], bias[:, 0:512], op=mybir.AluOpType.add
                )
                nc.vector.tensor_tensor(
                    ys[:, 512:768], pt[:, 512:768], bias[:, 512:768],
                    op=mybir.AluOpType.add,
                )
                nc.sync.dma_start(y_ext[n0 : n0 + 128, 0:512], ys[:, 0:512])
                nc.sync.dma_start(y_ext[n0 : n0 + 128, 512:768], ys[:, 512:768])

            aux_qs = {}  # nb -> staged q tile awaiting its second half

            def aux_q_job(nt_, nb, half):
                """q projection for n-tile nt_, block nb, e-half `half` through
                the 1-bank aux pool.  half 0 = e 0:512, half 1 = e 512:768."""
                w0, wid = (0, 512) if half == 0 else (512, 256)
                pa = auxp.tile([128, 512], F32, tag="aux", name="aq")
                n0 = nt_ * NTS + nb * 128
                for dc in range(DC):
                    nc.tensor.matmul(
                        pa[:, 0:wid],
                        xT[:, dc, n0 : n0 + 128],
                        wqq[:, dc, w0 : w0 + wid],
                        start=(dc == 0),
                        stop=(dc == DC - 1),
                    )
                if half == 0:
                    aux_qs[nb] = stage.tile(
                        [128, NHE, 128], BF16, tag=f"aqs{nb}", name="aqs", bufs=2
                    )
                qs = aux_qs[nb]
                if half == 0:
                    # heads 1..7 (psum cols 64:512) into slot tops 0..6, and
                    # the negated q_0 broadcast into all slot bottoms
                    nc.vector.tensor_copy(
                        qs[:, 0:7, 0:64],
                        pa[:, 64:512].rearrange("p (a b) -> p a b", a=7),
                    )
                    nc.vector.tensor_scalar_mul(
                        qs[:, :, 64:128],
                        pa[:, 0:64].unsqueeze(1).broadcast_to((128, NHE, 64)),
                        -1.0,
                    )
                else:
                    # heads 8..11 (psum cols 0:256 of the 512:768 half)
                    nc.vector.tensor_copy(
                        qs[:, 7:11, 0:64],
                        pa[:, 0:256].rearrange("p (a b) -> p a b", a=4),
                    )
                    nc.sync.dma_start_transpose(
                        qA[:, :, n0 : n0 + 128],
                        qs[:].rearrange("p a b -> p (a b)"),
                    )

            def aux_y_job(nt_, nb, half):
                """output projection y rows [nt_*256+nb*128, +128), dim-half."""
                w0, wid = (0, 512) if half == 0 else (512, 256)
                pa = auxp.tile([128, 512], F32, tag="aux", name="ay")
                n0 = nt_ * NTS + nb * 128
                for ec in range(DC):
                    nc.tensor.matmul(
                        pa[:, 0:wid],
                        outT[:, ec, n0 : n0 + 128],
                        wo[:, ec, w0 : w0 + wid],
                        start=(ec == 0),
                        stop=(ec == DC - 1),
                    )
                ys = ysg.tile([128, 512], F32, tag="ys", name="ys")
                nc.vector.tensor_tensor(
                    ys[:, 0:wid], pa[:, 0:wid], bias[:, w0 : w0 + wid],
                    op=mybir.AluOpType.add,
                )
                nc.sync.dma_start(y_ext[n0 : n0 + 128, w0 : w0 + wid], ys[:, 0:wid])

            prev = None  # (nt, mc, E, started) of previous chunk
            for nt in range(NT):
                for mc in range(MC):
                    if prev is None or prev[0] != nt:
                        # new n-tile: allocate fresh acc gens (waits on the
                        # previous nt's evac readers via tag cycling)
                        for q in range(3):
                            acc[q] = accp.tile(
                                [128, 8, 64], F32, tag=f"acc{q}", name=f"acc{q}"
                            )
                    E = epool.tile([128, NHE, NTS], BF16, tag="E", name="E")
                    for w, (s0, s1) in enumerate(WAVE_SLOTS):
                        nw = s1 - s0
                        sc = bigp.tile([128, 1024], F32, tag="big", name="sc")
                        for s in range(nw):
                            j = s0 + s
                            nc.tensor.matmul(
                                sc[:, s * 256 : (s + 1) * 256],
                                kA[:, j, mc * 128 : (mc + 1) * 128],
                                qA[:, j, nt * NTS : (nt + 1) * NTS],
                                start=True,
                                stop=True,
                            )
                        nc.scalar.activation(
                            E[:, s0:s1, :],
                            sc[:, 0 : nw * 256].rearrange("p (s n) -> p s n", s=nw),
                            mybir.ActivationFunctionType.Exp,
                            scale=float(SCALE),
                        )
                        if prev is not None:
                            emit_attnv_bank(prev, w)
                    # head-sum tree: DVE bf16 2x for the wide steps
                    t6 = small.tile([128, 6, NTS], BF16, tag="t6", name="t6")
                    t3 = small.tile([128, 3, NTS], BF16, tag="t3", name="t3")
                    S = small.tile([128, NTS], BF16, tag="S", name="S")
                    R = small.tile([128, NTS], BF16, tag="R", name="R")
                    add = mybir.AluOpType.add
                    if nt == NT - 1 and mc == MC - 1:
                        # final chunk: split by n-halves so n-block 0 retires
                        # with minimum latency (half A entirely on DVE, half B
                        # tree on Pool) -- this chain heads the drain sequence
                        ca, cb = slice(0, 128), slice(128, 256)
                        nc.vector.tensor_tensor(
                            t6[:, 0:5, ca], E[:, 0:5, ca], E[:, 5:10, ca], op=add
                        )
                        nc.vector.tensor_tensor(
                            t3[:, 0:2, ca], t6[:, 0:2, ca], t6[:, 2:4, ca], op=add
                        )
                        nc.vector.tensor_tensor(
                            t3[:, 2, ca], t3[:, 0, ca], t3[:, 1, ca], op=add
                        )
                        nc.vector.tensor_tensor(
                            t3[:, 2, ca], t3[:, 2, ca], t6[:, 4, ca], op=add
                        )
                        nc.vector.tensor_tensor(
                            t3[:, 2, ca], t3[:, 2, ca], E[:, 10, ca], op=add
                        )
                        nc.vector.tensor_scalar_add(S[:, ca], t3[:, 2, ca], 1.0)
                        with nc.allow_low_precision(reason="softmax denom"):
                            nc.vector.reciprocal(R[:, ca], S[:, ca])
                        nc.vector.tensor_mul(
                            E[:, :, ca],
                            E[:, :, ca],
                            R[:, ca].unsqueeze(1).broadcast_to((128, NHE, 128)),
                        )
                        nc.gpsimd.tensor_add(
                            t6[:, 0:5, cb], E[:, 0:5, cb], E[:, 5:10, cb]
                        )
                        nc.gpsimd.tensor_add(
                            t3[:, 0:2, cb], t6[:, 0:2, cb], t6[:, 2:4, cb]
                        )
                        nc.gpsimd.tensor_add(t3[:, 2, cb], t3[:, 0, cb], t3[:, 1, cb])
                        nc.gpsimd.tensor_add(t3[:, 2, cb], t3[:, 2, cb], t6[:, 4, cb])
                        nc.gpsimd.tensor_add(t3[:, 2, cb], t3[:, 2, cb], E[:, 10, cb])
                        nc.gpsimd.tensor_scalar_add(S[:, cb], t3[:, 2, cb], 1.0)
                        with nc.allow_low_precision(reason="softmax denom"):
                            nc.vector.reciprocal(R[:, cb], S[:, cb])
                        nc.gpsimd.tensor_mul(
                            E[:, :, cb],
                            E[:, :, cb],
                            R[:, cb].unsqueeze(1).broadcast_to((128, NHE, 128)),
                        )
                    else:
                        nc.vector.tensor_tensor(
                            t6[:, 0:5, :], E[:, 0:5, :], E[:, 5:10, :], op=add
                        )
                        nc.vector.tensor_tensor(
                            t3[:, 0:2, :], t6[:, 0:2, :], t6[:, 2:4, :], op=add
                        )
                        nc.gpsimd.tensor_add(t3[:, 2, :], t3[:, 0, :], t3[:, 1, :])
                        nc.gpsimd.tensor_add(t3[:, 2, :], t3[:, 2, :], t6[:, 4, :])
                        nc.gpsimd.tensor_add(t3[:, 2, :], t3[:, 2, :], E[:, 10, :])
                        nc.gpsimd.tensor_scalar_add(S[:], t3[:, 2, :], 1.0)
                        with nc.allow_low_precision(reason="softmax denom"):
                            nc.vector.reciprocal(R[:], S[:])
                        # second-to-last chunk: all muls on DVE so a greedy
                        # Pool-queue ordering can't head-of-line block the
                        # final attnv flush
                        nd = NHE if (nt == NT - 1 and mc == MC - 2) else MULS_DVE
                        nc.vector.tensor_mul(
                            E[:, 0:nd, :],
                            E[:, 0:nd, :],
                            R[:].unsqueeze(1).broadcast_to((128, nd, NTS)),
                        )
                        if nd < NHE:
                            nc.gpsimd.tensor_mul(
                                E[:, nd:NHE, :],
                                E[:, nd:NHE, :],
                                R[:].unsqueeze(1).broadcast_to(
                                    (128, NHE - nd, NTS)
                                ),
                            )

                    # overlapped phase-1 kv production (just-in-time mode)
                    if kv_jobs and nt == 0 and kv_jobs[0] <= mc + 2:
                        kv_block(kv_jobs.pop(0))

                    # aux-pool job: q for nt+1 at mc 0..3, y for nt-1 at mc 4..7
                    if mc < 4:
                        if nt + 1 < NT:
                            aux_q_job(nt + 1, mc // 2, mc % 2)
                    else:
                        if nt > 0:
                            aux_y_job(nt - 1, (mc - 4) // 2, mc % 2)

                    prev = (nt, mc, E, R)

                # end of n-tile: flush last chunk's attnv, evac accs
                if nt < NT - 1:
                    for w in range(3):
                        emit_attnv_bank(prev, w)
                    emit_acc_evac(nt)
                else:
                    # tail: nb-ordered drain, PE transposes, big-pool y
                    for nb in range(2):
                        for w in range(3):
                            emit_attnv_bank(prev, w, nbs=(nb,))
                        emit_tail_out_nb(nt, nb)
                        big_y_job(nt, nb)
                prev = None

    return nc


_NC_CACHE = {}


def _get_nc():
    key = (MULS_DVE, KV_HEAD)
    if key not in _NC_CACHE:
        _NC_CACHE[key] = build_nc()
    return _NC_CACHE[key]


def _bf16(a: np.ndarray) -> np.ndarray:
    return np.ascontiguousarray(a.astype(ml_dtypes.bfloat16))


def kernel(x, w_qkv, w_out, b_out):
    x = np.asarray(x, dtype=np.float32)
    w_qkv = _bf16(np.asarray(w_qkv, dtype=np.float32))
    w_out = _bf16(np.asarray(w_out, dtype=np.float32))
    b_out = np.asarray(b_out, dtype=np.float32)
    bias_bc = np.ascontiguousarray(np.broadcast_to(b_out[None, :], (128, DIM)))

    nc = _get_nc()
    in_maps = []
    for b in range(B):
        in_maps.append(
            {
                "xT": _bf16(x[b].T),
                "w_qkv": w_qkv,
                "w_out": w_out,
                "bias": bias_bc,
            }
        )
    res = run_bass_kernel_spmd(nc, in_maps, list(range(N_CORES)))
    y = np.stack([res.results[i]["y"] for i in range(N_CORES)], axis=0)
    return y
